# revision 42
# baseline (speedup 1.0000x reference)
"""MinGPT forward pass on 8 Trainium2 NeuronCores (Bass/Tile).

Sharding: core pair (2b, 2b+1) owns batch b. Within a pair, tensor
parallelism: core t owns attention heads t*8..t*8+7 and residual-stream
rows t*512..(t+1)*512 (feature-major [E, rows] layout on chip).

Per layer:  LN1 -> AllGather(x, 2 chunks) -> QKV fp8-DR (own heads, all
rows) -> causal attention bf16 (own heads, q-as-M AV with augmented-V
denominator) -> AllToAll(o, 2 chunks) -> h += o -> LN2 -> fc1 fp8-DR +
Gelu -> fc2 fp8-DR -> h += mlp.  Final LN + fp8-DR vocab head.

LayerNorm affine params are folded into the consuming weights on the
host (exact); on-chip LN is a pure normalize.  fp8 scale folding:
activations x FX, weights x FW, undone on the PSUM copy-out.
"""

import os
import sys

sys.path.insert(0, "/opt/trn_rl_repo")

import numpy as np
import ml_dtypes

import concourse.bass as bass
import concourse.bacc as bacc
import concourse.mybir as mybir
from concourse import tile
from concourse.bass_utils import run_bass_kernel_spmd

F32 = mybir.dt.float32
F32R = mybir.dt.float32r
BF16 = mybir.dt.bfloat16
FP8 = mybir.dt.float8e4
AF = mybir.ActivationFunctionType
OP = mybir.AluOpType
DRM = mybir.MatmulPerfMode.DoubleRow

B, S, E, H, D, L, V = 4, 1024, 1024, 16, 64, 12, 1024
NCORES = 8
ROWS = 512          # residual-stream rows owned per core
HL = 8              # heads per core
EPS = 1e-5
NEG = -3.0e38

FX = 1.0            # activation scale (1.0 for bf16)
FW = 1.0            # weight scale (1.0 for bf16)
PSQ = FX * FW       # psum scale of fp8 x@W matmuls
LOG_FX = float(np.log(FX))

RG = [[0, 1], [2, 3], [4, 5], [6, 7]]

LAST_EXEC_NS = None
LAST_RESULTS = None


def build_nc(has_bv=False, has_b2=False, n_layers=L):
    nc = bacc.Bacc(num_devices=NCORES)

    # ---- DRAM parameters (host pre-tiled, see kernel()) ----
    h0_d = nc.dram_tensor("h0", [128, 8 * ROWS], F32, kind="ExternalInput")
    wq_d = nc.dram_tensor("wq", [L, 128, 4096], BF16, kind="ExternalInput")
    wk_d = nc.dram_tensor("wk", [L, 128, 4096], BF16, kind="ExternalInput")
    wv_d = nc.dram_tensor("wv", [L, 128, 4096], BF16, kind="ExternalInput")
    bq_d = nc.dram_tensor("bq", [128, L * 4], F32, kind="ExternalInput")
    bk_d = nc.dram_tensor("bk", [128, L * 4], F32, kind="ExternalInput")
    bv_d = nc.dram_tensor("bv", [128, L * 8], F32, kind="ExternalInput")
    f1w_d = nc.dram_tensor("fc1w", [L, 128, 32768], BF16, kind="ExternalInput")
    f1b_d = nc.dram_tensor("fc1b", [128, L * 32], F32, kind="ExternalInput")
    f2w_d = nc.dram_tensor("fc2w", [L, 128, 32768], BF16, kind="ExternalInput")
    f2b_d = nc.dram_tensor("fc2b", [128, L * 8], F32, kind="ExternalInput")
    hw_d = nc.dram_tensor("headw", [128, 8192], BF16, kind="ExternalInput")
    msk_d = nc.dram_tensor("mask", [128, 128], F32, kind="ExternalInput")
    id_d = nc.dram_tensor("ident", [128, 128], BF16, kind="ExternalInput")
    hm_d = nc.dram_tensor("hmsk", [128, 2], F32, kind="ExternalInput")
    sel_d = nc.dram_tensor("sel", [8, 512], BF16, kind="ExternalInput")
    out_d = nc.dram_tensor("logits", [ROWS, V], F32, kind="ExternalOutput")
    dbga_d = nc.dram_tensor("dbg_a", [128, 8192], F32, kind="ExternalOutput")
    KDBG = os.environ.get("KDBG", "")

    with tile.TileContext(nc) as tc:
        with (
            tc.tile_pool(name="const", bufs=1) as cpool,
            tc.tile_pool(name="hres", bufs=1) as hpool,
            tc.tile_pool(name="act", bufs=1) as apool,
            tc.tile_pool(name="wgt", bufs=3) as wpool,
            tc.tile_pool(name="wv", bufs=1) as wvpool,
            tc.tile_pool(name="small", bufs=2) as spool,
            tc.tile_pool(name="tmp", bufs=2) as tpool,
            tc.tile_pool(name="exb", bufs=2) as epool,
            tc.tile_pool(name="orb", bufs=2) as opool,
            tc.tile_pool(name="avp", bufs=8) as avpool,
            # PSUM budget (8 banks): mm x3 + av x2 + bc x1 + st1/st2 x1
            tc.tile_pool(name="psmm", bufs=3, space="PSUM") as psmm,
            tc.tile_pool(name="psav", bufs=2, space="PSUM") as psav,
            tc.tile_pool(name="psbc", bufs=1, space="PSUM") as psbc,
            tc.tile_pool(name="psst", bufs=1, space="PSUM") as psst,
            tc.tile_pool(name="dram", bufs=2, space="DRAM") as dpool,
        ):
            # ---------- constants ----------
            ones_col = cpool.tile([128, 1], F32)
            nc.vector.memset(ones_col[:], 1.0)
            ones_bcol = cpool.tile([128, 1], BF16)
            nc.vector.memset(ones_bcol[:], 1.0)
            ones_row = cpool.tile([1, 128], BF16)
            nc.vector.memset(ones_row[:], 1.0)
            eps_c = cpool.tile([1, 1], F32)
            nc.vector.memset(eps_c[:], EPS)
            lfx_c = cpool.tile([1, 1], F32)
            nc.vector.memset(lfx_c[:], LOG_FX)
            mask_sb = cpool.tile([128, 128], F32)
            nc.sync.dma_start(mask_sb[:], msk_d[:])
            # one-hot selectors: sel[k, i*64+m] = (k == i), for broadcasting
            # row i of an [8, 512] tile to 64 partitions via matmul
            sel_sb = cpool.tile([8, 512], BF16)
            nc.sync.dma_start(sel_sb[:], sel_d[:])
            ident_sb = cpool.tile([128, 128], BF16)
            nc.sync.dma_start(ident_sb[:], id_d[:])
            hm_sb = cpool.tile([128, 2], F32)
            nc.sync.dma_start(hm_sb[:], hm_d[:])
            bq_sb = cpool.tile([128, L * 4], F32)
            nc.sync.dma_start(bq_sb[:], bq_d[:])
            bk_sb = cpool.tile([128, L * 4], F32)
            nc.sync.dma_start(bk_sb[:], bk_d[:])
            f1b_sb = cpool.tile([128, L * 32], F32)
            nc.sync.dma_start(f1b_sb[:], f1b_d[:])
            if has_b2:
                f2b_sb = cpool.tile([128, L * 8], F32)
                nc.sync.dma_start(f2b_sb[:], f2b_d[:])
            if has_bv:
                bv_sb = cpool.tile([128, L * 8], F32)
                nc.sync.dma_start(bv_sb[:], bv_d[:])

            # ---------- persistent tiles ----------
            h_all = hpool.tile([128, 8 * ROWS], F32)    # col et*512 + r
            nc.sync.dma_start(h_all[:], h0_d[:])

            def h_t(et):
                return h_all[:, et * ROWS:(et + 1) * ROWS]

            xn = apool.tile([128, 8, ROWS], BF16, name="xn")      # LN out (xFX)
            xf = apool.tile([128, 8, 1024], BF16, name="xf")      # gathered x
            q_all = apool.tile([128, 4096], BF16, name="q_all")  # col mq*1024+r
            k_all = apool.tile([128, 4096], BF16, name="k_all")
            v_aug = apool.tile([128, 8, 8, 65], BF16, name="v_aug")  # rt,h,f|1
            nc.vector.memset(v_aug[:, :, :, 64:65], 1.0)
            h1 = apool.tile([128, 32, ROWS], BF16, name="h1")     # mlp hidden

            # ---------- layernorm: stats -> broadcast A/C -> apply ----------
            def layer_norm(xdst):
                st1 = psst.tile([1, 512], F32, tag="st1")
                st2 = psst.tile([1, 512], F32, tag="st2")
                for et in range(8):
                    sq = tpool.tile([128, ROWS], BF16, tag="sq")
                    nc.scalar.activation(sq[:], h_t(et), AF.Square)
                    nc.tensor.matmul(st1[:], ones_col[:], h_t(et),
                                     start=(et == 0), stop=(et == 7))
                    nc.tensor.matmul(st2[:], ones_bcol[:], sq[:],
                                     start=(et == 0), stop=(et == 7))
                mean = spool.tile([1, 512], F32, tag="mean")
                nc.vector.tensor_scalar_mul(mean[:], st1[:], 1.0 / E)
                msq = spool.tile([1, 512], F32, tag="msq")
                nc.vector.tensor_scalar_mul(msq[:], st2[:], 1.0 / E)
                mean, msq = mean[:], msq[:]
                var = spool.tile([1, 512], F32, tag="var")
                nc.vector.tensor_mul(var[:], mean, mean)
                nc.vector.tensor_sub(var[:], msq, var[:])
                lnv = spool.tile([1, 512], F32, tag="lnv")
                nc.scalar.activation(lnv[:], var[:], AF.Ln, bias=eps_c[:])
                arow = spool.tile([1, 512], BF16, tag="arow")
                nc.scalar.activation(arow[:], lnv[:], AF.Exp, scale=-0.5)
                crow = spool.tile([1, 512], BF16, tag="crow")
                nc.vector.scalar_tensor_tensor(crow[:], mean, -1.0, arow[:],
                                               OP.mult, OP.mult)  # -mean*A
                a_ps = psbc.tile([128, 512], F32, tag="bc")
                nc.tensor.matmul(a_ps[:], ones_row[:], arow[:])
                a_sb = tpool.tile([128, 512], F32, tag="asb")
                nc.scalar.activation(a_sb[:], a_ps[:], AF.Identity)
                c_ps = psbc.tile([128, 512], F32, tag="bc")
                nc.tensor.matmul(c_ps[:], ones_row[:], crow[:])
                c_sb = tpool.tile([128, 512], F32, tag="csb")
                nc.scalar.activation(c_sb[:], c_ps[:], AF.Identity)
                for et in range(8):
                    eng = nc.vector if et % 2 == 0 else nc.gpsimd
                    t = tpool.tile([128, ROWS], F32, tag="lnap")
                    eng.tensor_mul(t[:], h_t(et), a_sb[:])
                    eng.tensor_add(xdst[:, et, :], t[:], c_sb[:])

            def dbg_dump(src_ap, ncols):
                d = tpool.tile([128, ncols], F32, tag="dbg")
                nc.vector.tensor_copy(d[:], src_ap)
                nc.sync.dma_start(dbga_d[:, 0:ncols], d[:])

            # ================= layers =================
            for l in range(n_layers):
                # ---- LN1 -> xn (fp8, x FX) ----
                layer_norm(xn)
                if l == 0 and KDBG == "xn":
                    dbg_dump(xn[:].rearrange("p a b -> p (a b)"), 4096)

                # ---- AllGather x across the pair, 2 chunks ----
                for ch in range(2):
                    cci = dpool.tile([512, 512], BF16, tag=f"cc1i{ch}")
                    cco = dpool.tile([1024, 512], BF16, tag=f"cc1o{ch}")
                    for e2 in range(4):
                        et = ch * 4 + e2
                        nc.sync.dma_start(cci[e2 * 128:(e2 + 1) * 128, :],
                                          xn[:, et, :])
                    nc.gpsimd.collective_compute(
                        "AllGather", OP.bypass, replica_groups=RG,
                        ins=[cci[:].opt()], outs=[cco[:].opt()])
                    for e2 in range(4):
                        et = ch * 4 + e2
                        for rk in range(2):
                            nc.sync.dma_start(
                                xf[:, et, rk * 512:(rk + 1) * 512],
                                cco[rk * 512 + e2 * 128:
                                    rk * 512 + (e2 + 1) * 128, :])
                if l == 0 and KDBG == "xf":
                    dbg_dump(xf[:].rearrange("p a b -> p (a b)"), 8192)

                # ---- QKV (own 8 heads, all 1024 rows), fp8 DoubleRow ----
                for (w_d, b_sb, dst) in ((wq_d, bq_sb, q_all), (wk_d, bk_sb, k_all)):
                    for mq in range(4):
                        wt = wpool.tile([128, 8, 128], BF16, tag="wqk")
                        nc.sync.dma_start(wt[:], w_d[l][:, mq * 1024:(mq + 1) * 1024])
                        for nb in range(2):
                            ps = psmm.tile([128, 512], F32, tag="mm")
                            for j in range(8):
                                nc.tensor.matmul(
                                    ps[:], wt[:, j, :],
                                    xf[:, j, nb * 512:(nb + 1) * 512],
                                    start=(j == 0), stop=(j == 7))
                            nc.scalar.activation(
                                dst[:, mq * 1024 + nb * 512:
                                    mq * 1024 + nb * 512 + 512],
                                ps[:], AF.Identity, scale=1.0 / PSQ,
                                bias=b_sb[:, l * 4 + mq:l * 4 + mq + 1])
                # V row-major [1024r, 8h x (64f|1)]
                wv_sb = wvpool.tile([128, 8, 512], BF16, tag="wv")
                nc.sync.dma_start(wv_sb[:], wv_d[l][:])
                for rt in range(8):
                    ps = psmm.tile([128, 8, 64], F32, tag="mm")
                    for j in range(8):
                        nc.tensor.matmul(
                            ps[:], xf[:, j, rt * 128:(rt + 1) * 128],
                            wv_sb[:, j, :],
                            start=(j == 0), stop=(j == 7))
                    nc.scalar.activation(v_aug[:, rt, :, 0:64], ps[:],
                                         AF.Identity, scale=1.0 / PSQ)

                if l == 0 and KDBG == "q":
                    dbg_dump(q_all[:], 4096)
                if l == 0 and KDBG == "k":
                    dbg_dump(k_all[:], 4096)
                if l == 0 and KDBG == "v":
                    dbg_dump(v_aug[:].rearrange("p a b c -> p (a b c)"), 4160)

                # ---- causal attention, own 8 heads (bf16) ----
                # AV keeps v stationary: out [65, 512] per (head, qg) with
                # the denominator in partition-row 64 (augmented-V ones
                # column).  Causal narrowing uses nested PSUM regions
                # [off..512) accumulated kt-ascending.  Denominators for a
                # 4-head chunk are batched into one [8, 512] reciprocal;
                # normalization is (av * hm) * bcast(rden) fused in one stt.
                # o exchange staging (feature-major): rows = dest*512 +
                # tb*256 + chunk-local feature, cols = q_local; each core
                # writes both tb blocks (own: x1, other: x0 via hmsk), so
                # the pair ReduceScatter concats feature halves.
                cc2i = [dpool.tile([1024, 512], BF16, tag=f"cc2i{c}",
                                   name=f"cc2i{c}") for c in range(2)]
                cc2o = [dpool.tile([512, 512], BF16, tag=f"cc2o{c}",
                                   name=f"cc2o{c}") for c in range(2)]
                for c in range(2):
                    den_all = spool.tile([8, 512], F32, tag="den",
                                         name="den_all")
                    avs = []
                    for h2 in range(4):
                        hh = c * 4 + h2
                        hp, ho = hh // 2, (hh % 2) * 64
                        q_ap = q_all[ho:ho + 64, hp * 1024:(hp + 1) * 1024]
                        k_ap = k_all[ho:ho + 64, hp * 1024:(hp + 1) * 1024]
                        for qg in range(2):
                            nkt = 4 * qg + 4
                            av = psav.tile([65, 512], F32, tag="av",
                                           name=f"av{h2}{qg}")
                            for kt in range(nkt):
                                dg = kt - 4 * qg
                                off = max(dg, 0) * 128
                                sc = psmm.tile([128, 512], F32, tag="mm")
                                nc.tensor.matmul(
                                    sc[:, 0:512 - off],
                                    k_ap[:, kt * 128:(kt + 1) * 128],
                                    q_ap[:, qg * 512 + off:(qg + 1) * 512])
                                if dg >= 0:
                                    nc.vector.tensor_add(
                                        sc[:, 0:128], sc[:, 0:128], mask_sb[:])
                                ex = epool.tile([128, 512], BF16, tag="ex")
                                nc.scalar.activation(
                                    ex[:, 0:512 - off], sc[:, 0:512 - off],
                                    AF.Exp, scale=0.125)
                                nc.tensor.matmul(
                                    av[:, off:512], v_aug[:, kt, hh, :],
                                    ex[:, 0:512 - off],
                                    start=(kt == 0), stop=(kt == nkt - 1),
                                    skip_group_check=True)
                            den_sb = spool.tile([1, 512], F32, tag="densb")
                            nc.scalar.activation(den_sb[:], av[64:65, :],
                                                 AF.Identity)
                            nc.sync.dma_start(
                                den_all[h2 * 2 + qg:h2 * 2 + qg + 1, :],
                                den_sb[:])
                            av_sb = avpool.tile([64, 512], BF16, tag="avsb")
                            nc.scalar.activation(av_sb[:], av[0:64, :],
                                                 AF.Identity)
                            avs.append((hh, qg, av_sb))
                    rden = spool.tile([8, 512], BF16, tag="rden")
                    with nc.allow_low_precision(reason="bf16 softmax rden"):
                        nc.vector.reciprocal(rden[:], den_all[:])
                    for i, (hh, qg, av_sb) in enumerate(avs):
                        rb = psbc.tile([64, 512], F32, tag="bc")
                        nc.tensor.matmul(rb[:], sel_sb[:, i * 64:(i + 1) * 64],
                                         rden[:])
                        for tb in range(2):
                            om = spool.tile([64, 512], BF16, tag=f"om{tb}",
                                            name=f"om{tb}")
                            nc.vector.scalar_tensor_tensor(
                                om[:], av_sb[:], hm_sb[0:64, tb:tb + 1],
                                rb[:], OP.mult, OP.mult)
                            nc.sync.dma_start(
                                cc2i[c][qg * 512 + tb * 256 + (hh % 4) * 64:
                                        qg * 512 + tb * 256 + (hh % 4) * 64
                                        + 64, :],
                                om[:])
                    nc.gpsimd.collective_compute(
                        "ReduceScatter", OP.add, replica_groups=RG,
                        ins=[cc2i[c][:].opt()], outs=[cc2o[c][:].opt()])

                # readback: chunk c rows = tb*256 + fl -> et = tb*4+c*2+fl/128
                for c in range(2):
                    for tb in range(2):
                        for i in range(2):
                            et = tb * 4 + c * 2 + i
                            ot = opool.tile([128, 512], BF16, tag="ot")
                            nc.sync.dma_start(
                                ot[:], cc2o[c][tb * 256 + i * 128:
                                               tb * 256 + (i + 1) * 128, :])
                            of = opool.tile([128, 512], F32, tag="of")
                            nc.gpsimd.tensor_copy(of[:], ot[:])
                            nc.vector.tensor_add(h_t(et), h_t(et), of[:])

                if has_bv:
                    # softmax weights sum to 1, so the V bias passes through
                    # attention unchanged: h += bv (full-width, per-feature).
                    for et in range(8):
                        nc.vector.tensor_scalar_add(
                            h_t(et), h_t(et),
                            bv_sb[:, l * 8 + et:l * 8 + et + 1])

                if l == 0 and KDBG == "hattn":
                    dbg_dump(h_all[:], 4096)

                # ---- LN2 -> xn (reuse buffer) ----
                layer_norm(xn)

                # ---- fc1 + gelu (fp8 DR), full hidden, own 512 rows ----
                for mh in range(32):
                    wt = wpool.tile([128, 8, 128], BF16, tag="wf1")
                    nc.sync.dma_start(wt[:], f1w_d[l][:, mh * 1024:(mh + 1) * 1024])
                    ps = psmm.tile([128, 512], F32, tag="mm")
                    for j in range(8):
                        nc.tensor.matmul(
                            ps[:], wt[:, j, :], xn[:, j, :],
                            start=(j == 0), stop=(j == 7))
                    nc.scalar.activation(
                        h1[:, mh, :], ps[:], AF.Gelu,
                        bias=f1b_sb[:, l * 32 + mh:l * 32 + mh + 1])

                if l == 0 and KDBG == "h1":
                    dbg_dump(h1[:, 0:16, :].rearrange("p a b -> p (a b)"), 8192)

                # ---- fc2 (fp8 DR) + residual ----
                for mo in range(8):
                    ps = psmm.tile([128, 512], F32, tag="mm")
                    for half in range(2):
                        wt = wpool.tile([128, 16, 128], BF16, tag="wf2")
                        nc.sync.dma_start(
                            wt[:], f2w_d[l][:, mo * 4096 + half * 2048:
                                            mo * 4096 + (half + 1) * 2048])
                        for j in range(16):
                            kg = half * 16 + j
                            nc.tensor.matmul(
                                ps[:], wt[:, j, :], h1[:, kg, :],
                                start=(kg == 0), stop=(kg == 31))
                    if has_b2:
                        t = tpool.tile([128, 512], F32, tag="f2o")
                        nc.vector.tensor_scalar(
                            t[:], ps[:], 1.0 / FW,
                            f2b_sb[:, l * 8 + mo:l * 8 + mo + 1],
                            OP.mult, OP.add)
                        nc.vector.tensor_add(h_t(mo), h_t(mo), t[:])
                    else:
                        nc.vector.scalar_tensor_tensor(
                            h_t(mo), ps[:], 1.0 / FW, h_t(mo),
                            OP.mult, OP.add)

                if l == 0 and KDBG == "hlayer":
                    dbg_dump(h_all[:], 4096)

            # ================= final LN + head =================
            layer_norm(xn)
            for vn in range(2):
                hw_sb = wvpool.tile([128, 8, 512], BF16, tag="wv")
                nc.sync.dma_start(
                    hw_sb[:],
                    hw_d[:].rearrange("p (a b) -> p a b", a=8)[:, :,
                                      vn * 512:(vn + 1) * 512])
                for rt in range(4):
                    ps = psmm.tile([128, 512], F32, tag="mm")
                    for j in range(8):
                        nc.tensor.matmul(
                            ps[:],
                            xn[:, j, rt * 128:rt * 128 + 128],
                            hw_sb[:, j, :],
                            start=(j == 0), stop=(j == 7))
                    lt = tpool.tile([128, 512], F32, tag="lt")
                    nc.scalar.activation(lt[:], ps[:], AF.Identity)
                    nc.sync.dma_start(out_d[rt * 128:(rt + 1) * 128,
                                            vn * 512:(vn + 1) * 512], lt[:])

    nc.finalize()
    return nc


# ---------------------------------------------------------------------------
#  Host side: fold LN params, shard/pre-tile inputs, run, gather
# ---------------------------------------------------------------------------

def _tile_lhsT(w):
    """(K, M) -> [128, (M/128)*(K/128)*128]: col = mi*K + et*128 + m."""
    Kdim, Mdim = w.shape
    kt, mt = Kdim // 128, Mdim // 128
    w4 = w.reshape(kt, 128, mt, 128)
    out = np.empty((128, mt * kt * 128), dtype=w.dtype)
    for mi in range(mt):
        blk = np.transpose(w4[:, :, mi, :], (1, 0, 2)).reshape(128, kt * 128)
        out[:, mi * kt * 128:(mi + 1) * kt * 128] = blk
    return out


def _tile_rhs(w):
    """(K, N) -> [128, (K/128)*N] with col = et*N + n."""
    Kdim, Ndim = w.shape
    kt = Kdim // 128
    return np.ascontiguousarray(
        np.transpose(w.reshape(kt, 128, Ndim), (1, 0, 2)).reshape(128, kt * Ndim))


def _tile_vec(v, blk=128):
    """(L?, F) with F=ft*128 -> [128, L*ft] col = l*ft + et."""
    if v.ndim == 1:
        v = v[None, :]
    Ldim, F = v.shape
    ft = F // blk
    return np.ascontiguousarray(
        np.transpose(v.reshape(Ldim, ft, blk), (2, 0, 1)).reshape(blk, Ldim * ft))


def _fp8(w):
    # bf16 weight cast (name kept from the fp8 experiment)
    return np.asarray(w, np.float32).astype(ml_dtypes.bfloat16)


def kernel(tokens, tok_emb, pos_emb, ln_w, ln_b, qkv_w, qkv_b,
           fc1_w, fc1_b, fc2_w, fc2_b, lnf_w, lnf_b, head_w):
    global LAST_EXEC_NS, LAST_RESULTS
    f32 = np.float32
    tokens = np.asarray(tokens)
    tok_emb = np.asarray(tok_emb, f32)
    pos_emb = np.asarray(pos_emb, f32)
    ln_w = np.asarray(ln_w, f32); ln_b = np.asarray(ln_b, f32)
    qkv_w = np.asarray(qkv_w, f32); qkv_b = np.asarray(qkv_b, f32)
    fc1_w = np.asarray(fc1_w, f32); fc1_b = np.asarray(fc1_b, f32)
    fc2_w = np.asarray(fc2_w, f32); fc2_b = np.asarray(fc2_b, f32)
    lnf_w = np.asarray(lnf_w, f32); lnf_b = np.asarray(lnf_b, f32)
    head_w = np.asarray(head_w, f32)

    # embedding on host (0.1% of model FLOPs)
    emb = tok_emb[tokens.astype(np.int64)] + pos_emb[None, :S, :]   # [B,S,E]

    # fold LN affine params into consuming weights (exact)
    qkv_w_eff = qkv_w * ln_w[:, :, None]
    qkv_b_eff = qkv_b + np.einsum('le,lef->lf', ln_b, qkv_w)
    fc1_w_eff = fc1_w * ln_w[:, :, None]
    fc1_b_eff = fc1_b + np.einsum('le,lef->lf', ln_b, fc1_w)
    head_w_eff = head_w * lnf_w[:, None]
    logit_bias = lnf_b @ head_w                                     # [V]

    has_b2 = bool(np.any(fc2_b != 0.0))
    bv_all = qkv_b_eff[:, 2 * E:3 * E]
    has_bv = bool(np.any(bv_all != 0.0))

    # causal mask for the 128x128 diagonal blocks of scores^T [kv, q]
    p = np.arange(128)[:, None]
    c = np.arange(128)[None, :]
    mask = np.where(p <= c, 0.0, NEG).astype(f32)
    ident = np.eye(128, dtype=ml_dtypes.bfloat16)

    in_maps = []
    for core in range(NCORES):
        b, t = core // 2, core % 2
        hs = t * 8          # first head
        wq = qkv_w_eff[:, :, hs * D:(hs + 8) * D]                 # [L,1024,512]
        wk = qkv_w_eff[:, :, E + hs * D: E + (hs + 8) * D]
        wv = qkv_w_eff[:, :, 2 * E + hs * D: 2 * E + (hs + 8) * D]
        bq = qkv_b_eff[:, hs * D:(hs + 8) * D]
        bk = qkv_b_eff[:, E + hs * D:E + (hs + 8) * D]

        h0 = emb[b, t * ROWS:(t + 1) * ROWS, :].T                 # [1024, 512]

        im = {
            "h0": _tile_rhs(np.ascontiguousarray(h0)).astype(f32),
            "wq": np.stack([_fp8(_tile_lhsT(wq[l])) for l in range(L)]),
            "wk": np.stack([_fp8(_tile_lhsT(wk[l])) for l in range(L)]),
            "wv": np.stack([_fp8(_tile_rhs(wv[l])) for l in range(L)]),
            "bq": _tile_vec(bq).astype(f32),
            "bk": _tile_vec(bk).astype(f32),
            "bv": _tile_vec(bv_all).astype(f32),
            "fc1w": np.stack([_fp8(_tile_lhsT(fc1_w_eff[l])) for l in range(L)]),
            "fc1b": _tile_vec(fc1_b_eff).astype(f32),
            "fc2w": np.stack([_fp8(_tile_lhsT(fc2_w[l])) for l in range(L)]),
            "fc2b": _tile_vec(fc2_b).astype(f32),
            "headw": _fp8(_tile_rhs(head_w_eff)),
            "mask": mask,
            "ident": ident,
            "hmsk": np.ascontiguousarray(np.broadcast_to(
                np.eye(2, dtype=f32)[t][None, :], (128, 2))),
            "sel": np.repeat(np.eye(8, dtype=ml_dtypes.bfloat16), 64, axis=1),
        }
        in_maps.append(im)

    nc = build_nc(has_bv=has_bv, has_b2=has_b2)
    res = run_bass_kernel_spmd(
        nc, in_maps, core_ids=list(range(NCORES)),
        trace=bool(int(os.environ.get("KTRACE", "0"))))
    LAST_EXEC_NS = res.exec_time_ns
    LAST_RESULTS = res

    out = np.empty((B, S, V), f32)
    for core in range(NCORES):
        b, t = core // 2, core % 2
        out[b, t * ROWS:(t + 1) * ROWS, :] = res.results[core]["logits"]
    if np.any(logit_bias != 0.0):
        out += logit_bias[None, None, :]
    return out
